# revision 47
# baseline (speedup 1.0000x reference)
"""AttentionSinkPrefill Trainium2 kernel v2 (8 NeuronCores, sequence-parallel).

Module:   Y = AttnSinkPrefill(X) with sink=4, window=256, causal GQA
          (16 q heads, 4 kv heads, head_dim 64, d_model 1024, B=2, T=2048).

Sharding: sequence-parallel over T.  Core c handles queries
          [256c, 256c+256) for both batches; needs X rows
          [256c-256, 256c+256) (zero-padded left) plus the 4 sink rows.
          No collectives; outputs concatenate.

v2 changes vs v1 (301779ns / 253869ns baseline):
  - bf16 matmul operands everywhere (measured rel err 3.8e-3, gate 2e-2)
  - DMA issue order: X window tiles for batch 0 land first; weights and
    Wo stream behind compute (v1 stalled the PE 34us on weight DMAs)
  - block-sparse attention over 128x128 diagonal blocks: per 256-query
    half only 3 of 4 key blocks are live; the 2 structurally-empty
    blocks are never computed, full blocks skip masking entirely
  - 4 q-heads sharing a kv head are processed per iteration with N=512
    matmuls via strided rhs views into one merged Q tile (4x fewer
    stationary-weight loads in attention)
  - per-core triangular block masks + fine-grained sink mask keep the
    program uniform across cores; zero "ones-columns" kill padding-key
    denominator contributions on boundary cores
  - b1 projections / O-proj are interleaved as PE filler between
    attention iterations so the tensor engine stays continuously busy
    (TRN2 PE only reaches 2.4GHz after 3us without gaps; idle drops it
    to 1.2GHz)
"""

import os
import sys
from contextlib import ExitStack

import numpy as np

sys.path.insert(0, "/opt/trn_rl_repo")

import concourse.bass as bass
import concourse.bacc as bacc
import concourse.mybir as mybir
import concourse.tile as tile
from concourse.bass_utils import run_bass_kernel_spmd

# ---------------------------------------------------------------- constants
D = 1024          # d_model
NH = 16           # q heads
NKV = 4           # kv heads
HD = 64           # head dim
SINK = 4          # attention sink width
WIN = 256         # sliding window
B = 2
T = 2048
NCORES = 8
QB = T // NCORES  # queries per core = 256
KW = 2 * QB       # window key rows per core = 512
KCOL = KW + SINK  # 516 key columns per batch in X^T layout

F32 = mybir.dt.float32
FR = mybir.dt.float32r
MM = mybir.dt.bfloat16
AF = mybir.ActivationFunctionType
MULT = mybir.AluOpType.mult

# head order placing each q head at partition base (kv_head%2)*64, with the
# 4 q heads of kv head g in consecutive m-tiles 4*(g//2)..+4 at that base;
# Wq columns / Wo rows are permuted to match.
HEAD_ORDER = [0, 4, 1, 5, 2, 6, 3, 7, 8, 12, 9, 13, 10, 14, 11, 15]

# attention blocks per (g, b) iteration: (tag, tki, chalf, mask_id)
#   mask_id: 0-3 -> per-core triangular mask tile, 'snk' -> sink mask, None -> full
S_BLOCKS = [
    ("P0", 0, 0, 0),
    ("P1a", 1, 0, None),
    ("P1b", 1, 1, 1),
    ("P2a", 2, 0, 2),
    ("P2b", 2, 1, None),
    ("P3", 3, 1, 3),
    ("S0a", None, 0, "snk"),   # sink keys, c0
    ("S0b", None, 1, "snk"),   # sink keys, c1
]
# PV accumulation target (0 -> ysA/c0, 1 -> ysB/c1) per block follows chalf.


# ================================================================ program
def build_nc():
    nc = bacc.Bacc()

    xw_d = nc.dram_tensor("Xw", [B, KW, D], MM, kind="ExternalInput")
    xs_d = nc.dram_tensor("Xs", [B, SINK, D], MM, kind="ExternalInput")
    idt_d = nc.dram_tensor("IDT", [128, 128], MM, kind="ExternalInput")
    wq_d = nc.dram_tensor("Wq", [D, NH * HD], MM, kind="ExternalInput")
    wk_d = nc.dram_tensor("Wk", [D, NKV * HD], MM, kind="ExternalInput")
    wv_d = nc.dram_tensor("Wv", [D, NKV * HD], MM, kind="ExternalInput")
    wo_d = nc.dram_tensor("Wo", [NH * HD, D], MM, kind="ExternalInput")
    mp_d = nc.dram_tensor("MP", [4, 128, 4 * 128], MM, kind="ExternalInput")
    msnk_d = nc.dram_tensor("MSNK", [SINK, 2 * 512], MM, kind="ExternalInput")
    onew_d = nc.dram_tensor("ONEW", [4, 128, NKV], MM, kind="ExternalInput")
    onb_d = nc.dram_tensor("ONB", [65, 64], FR, kind="ExternalInput")
    out_d = nc.dram_tensor("out", [B, QB, D], F32, kind="ExternalOutput")

    with nc.allow_low_precision(reason="bf16 matmul operands"), \
            tile.TileContext(nc) as tc, ExitStack() as ctx:
        cst = ctx.enter_context(tc.tile_pool(name="cst", bufs=1))
        wts = ctx.enter_context(tc.tile_pool(name="wts", bufs=1))
        wop = ctx.enter_context(tc.tile_pool(name="wop", bufs=1))
        xtp = ctx.enter_context(tc.tile_pool(name="xtp", bufs=1))
        qkv = ctx.enter_context(tc.tile_pool(name="qkv", bufs=1))
        ytp = ctx.enter_context(tc.tile_pool(name="ytp", bufs=1))
        ppool = ctx.enter_context(tc.tile_pool(name="pp", bufs=2))
        misc = ctx.enter_context(tc.tile_pool(name="misc", bufs=2))
        # 3 x [128,1024] (2 banks each) ring shared by projections and the
        # attention S-stream + 2 x [65,512] (1 bank) ys accumulators = 8 banks
        psP = ctx.enter_context(tc.tile_pool(name="psP", bufs=3, space="PSUM"))
        psY = ctx.enter_context(tc.tile_pool(name="psY", bufs=2, space="PSUM"))

        # ------------------------------------------------ persistent tiles
        # X^T per batch: [128, (d-tile, key)], built by all-bf16 PE
        # transposes (1 cycle/row) from bf16 X tiles
        xtb = [xtp.tile([128, 8 * KCOL], MM, tag=f"xtb{b}", name=f"xtb{b}")
               for b in range(B)]

        def xt(b, d):
            return xtb[b][:, d * KCOL:d * KCOL + KCOL]

        # ------------------------------------------------ prologue DMAs
        # X tiles first (the PE's first work), weights stream behind
        ident = cst.tile([128, 128], MM, tag="idt")
        nc.sync.dma_start(ident[:], idt_d[:])
        xws = {}
        for b in range(B):
            for tki in range(4):
                t = xtp.tile([128, D], MM, tag=f"xw{tki}_{b}",
                             name=f"xw{tki}_{b}")
                xws[(b, tki)] = t
            t = xtp.tile([SINK, D], MM, tag=f"xs_{b}", name=f"xs_{b}")
            xws[(b, "s")] = t
        for b in range(B):
            for tki in range(4):
                nc.sync.dma_start(
                    xws[(b, tki)][:], xw_d[b, tki * 128:(tki + 1) * 128, :])
            nc.sync.dma_start(xws[(b, "s")][:], xs_d[b])

        wk = []
        wv = []
        for d in range(8):
            t = wts.tile([128, NKV * HD], MM, tag=f"wk{d}", name=f"wk{d}")
            nc.gpsimd.dma_start(t[:], wk_d[d * 128:(d + 1) * 128, :])
            wk.append(t)
            t = wts.tile([128, NKV * HD], MM, tag=f"wv{d}", name=f"wv{d}")
            nc.gpsimd.dma_start(t[:], wv_d[d * 128:(d + 1) * 128, :])
            wv.append(t)

        wq = []
        for d in range(8):
            t = wts.tile([128, NH * HD], MM, tag=f"wq{d}", name=f"wq{d}")
            nc.gpsimd.dma_start(t[:], wq_d[d * 128:(d + 1) * 128, :])
            wq.append(t)

        mp = []
        for i in range(4):
            t = cst.tile([128, 512], MM, tag=f"mp{i}", name=f"mp{i}")
            nc.gpsimd.dma_start(t[:], mp_d[i])
            mp.append(t)
        msnk = cst.tile([SINK, 1024], MM, tag="msnk")
        nc.gpsimd.dma_start(msnk[:], msnk_d[:])
        onb = cst.tile([65, 64], FR, tag="onb")
        nc.gpsimd.dma_start(onb[:], onb_d[:])

        wo = []
        for m in range(8):
            t = wop.tile([128, D], MM, tag=f"wo{m}", name=f"wo{m}")
            nc.gpsimd.dma_start(t[:], wo_d[m * 128:(m + 1) * 128, :])
            wo.append(t)
        qt = qkv.tile([128, 8 * B * QB], MM, tag="qt")  # [128,(m,b,q)] merged
        kt = [qkv.tile([128, B * KW], MM, tag=f"kt{m}", name=f"kt{m}")
              for m in range(2)]
        ktp = {}
        for mk in range(2):
            for b in range(B):
                t = qkv.tile([128, 128], MM, tag=f"ktp{mk}{b}", name=f"ktp{mk}{b}")
                nc.gpsimd.memset(t[:], 0.0)
                ktp[(mk, b)] = t
        vt = {}
        for tki in range(4):
            for b in range(B):
                t = qkv.tile([128, NKV * (HD + 1)], MM,
                              tag=f"vt{tki}{b}", name=f"vt{tki}{b}")
                nc.sync.dma_start(t[:, HD:NKV * (HD + 1):HD + 1], onew_d[tki])
                vt[(tki, b)] = t
        vs = {}
        for b in range(B):
            t = qkv.tile([SINK, NKV * (HD + 1)], MM, tag=f"vs{b}", name=f"vs{b}")
            nc.gpsimd.memset(t[0:SINK, HD:NKV * (HD + 1):HD + 1], 1.0)
            vs[b] = t
        yt = ytp.tile([128, 8 * B * QB], MM, tag="yt")  # [128,(m,b,q)]

        # ------------------------------------------------ emit helpers
        def emit_transposes(b):
            # tki-outer so the first transpose only needs the first X tile;
            # all-bf16: 1 cycle/row on the PE, 2x-mode copies.  X^T copies
            # go to ACT for b0 (idle prologue) and DVE for b1 (filler time)
            for tki in range(4):
                ps = psP.tile([128, 2048], MM, tag="s", name=f"trp{b}{tki}")
                for d in range(8):
                    nc.tensor.transpose(
                        ps[:, d * 128:(d + 1) * 128],
                        xws[(b, tki)][:, d * 128:(d + 1) * 128],
                        ident[:],
                    )
                for d in range(8):
                    dst = xt(b, d)[:, tki * 128:(tki + 1) * 128]
                    src = ps[:, d * 128:(d + 1) * 128]
                    if b == 0:
                        nc.scalar.copy(dst, src)
                    else:
                        nc.vector.tensor_copy(dst, src)
            # sink rows: 8 transposes [128,4] packed into one psum tile
            ps = psP.tile([128, 2048], MM, tag="s", name=f"trs{b}")
            for d in range(8):
                nc.tensor.transpose(
                    ps[:, d * 4:d * 4 + SINK],
                    xws[(b, "s")][0:SINK, d * 128:(d + 1) * 128],
                    ident[0:SINK, 0:SINK],
                )
            for d in range(8):
                nc.vector.tensor_copy(
                    xt(b, d)[:, KW:KCOL],
                    ps[:, d * 4:d * 4 + SINK],
                )

        def emit_K(b):
            for mk in range(2):
                ps = psP.tile([128, 1024], F32, tag="s", name=f"kps{b}{mk}")
                for d in range(8):
                    nc.tensor.matmul(
                        ps[:, 0:KW],
                        wk[d][:, mk * 128:(mk + 1) * 128],
                        xt(b, d)[:, 0:KW],
                        start=(d == 0), stop=(d == 7),
                    )
                for d in range(8):
                    nc.tensor.matmul(
                        ps[:, KW:KW + SINK],
                        wk[d][:, mk * 128:(mk + 1) * 128],
                        xt(b, d)[:, KW:KCOL],
                        start=(d == 0), stop=(d == 7),
                    )
                nc.vector.tensor_copy(
                    kt[mk][:, b * KW:(b + 1) * KW], ps[:, 0:KW])
                nc.vector.tensor_copy(ktp[(mk, b)][:, 0:SINK], ps[:, KW:KW + SINK])

        def emit_V(b):
            for tki in range(4):
                ps = psP.tile([128, 1024], F32, tag="s", name=f"vps{b}{tki}")
                for d in range(8):
                    nc.tensor.matmul(
                        ps[:, 0:NKV * HD],
                        xt(b, d)[:, tki * 128:(tki + 1) * 128],
                        wv[d][:],
                        start=(d == 0), stop=(d == 7),
                    )
                nc.vector.tensor_copy(
                    vt[(tki, b)][:].rearrange(
                        "p (g c) -> p g c", g=NKV)[:, :, 0:HD],
                    ps[:, 0:NKV * HD].rearrange("p (g c) -> p g c", g=NKV),
                )
            ps = psP.tile([128, 1024], F32, tag="s", name=f"vsps{b}")
            for d in range(8):
                nc.tensor.matmul(
                    ps[0:SINK, 0:NKV * HD],
                    xt(b, d)[:, KW:KCOL],
                    wv[d][:],
                    start=(d == 0), stop=(d == 7),
                )
            nc.vector.tensor_copy(
                vs[b][0:SINK].rearrange("p (g c) -> p g c", g=NKV)[:, :, 0:HD],
                ps[0:SINK, 0:NKV * HD].rearrange("p (g c) -> p g c", g=NKV),
            )

        def emit_Q(b):
            # two head-tiles per psum tile, one strided copy into qt
            for j in range(4):
                ps = psP.tile([128, 1024], F32, tag="s", name=f"qps{b}{j}")
                for mi in range(2):
                    m = 2 * j + mi
                    for d in range(8):
                        nc.tensor.matmul(
                            ps[:, mi * QB:(mi + 1) * QB],
                            wq[d][:, m * 128:(m + 1) * 128],
                            xt(b, d)[:, KW - QB:KW],
                            start=(d == 0), stop=(d == 7),
                        )
                nc.vector.tensor_copy(
                    qt[:].rearrange("p (m b q) -> p m b q", m=8, b=B)
                      [:, 2 * j:2 * j + 2, b:b + 1, :],
                    ps[:, 0:2 * QB].rearrange("p (m b q) -> p m b q", m=2, b=1),
                )

        # attention state carried across emit calls
        sps = {}    # (g,b) -> list of S psum tiles (same order as S_BLOCKS)
        ys = {}     # (g,b) -> (ysA, ysB)
        ptile = {}  # (g,b) -> p tile

        def qview(g, b, c):
            kb = (g % 2) * 64
            m0 = 4 * (g // 2)
            return qt[kb:kb + 64].rearrange(
                "p (m b q) -> p m b q", m=8, b=B
            )[:, m0:m0 + 4, b:b + 1, c * 128:(c + 1) * 128]

        def emit_attn_S(g, b):
            # 8 S matmuls into 4 two-bank psum tiles; one exp per pair
            kb = (g % 2) * 64
            mk = g // 2
            p = ppool.tile([128, 8 * 512], MM, tag="p", name=f"p{g}{b}")
            ptile[(g, b)] = p
            for pair in range(4):
                sp = psP.tile([128, 1024], F32, tag="s", name=f"s{g}{b}{pair}")
                for half in range(2):
                    bi = pair * 2 + half
                    tag, tki, c, _m = S_BLOCKS[bi]
                    if tki is None:
                        lhsT = ktp[(mk, b)][kb:kb + 64, :]
                    else:
                        lhsT = kt[mk][kb:kb + 64,
                                      b * KW + tki * 128:b * KW + (tki + 1) * 128]
                    nc.tensor.matmul(sp[:, half * 512:(half + 1) * 512],
                                     lhsT, qview(g, b, c),
                                     start=True, stop=True)
                nc.scalar.activation(
                    p[:, pair * 1024:(pair + 1) * 1024], sp[:], AF.Exp)
                for half in range(2):
                    bi = pair * 2 + half
                    tag, tki, c, mid = S_BLOCKS[bi]
                    if mid is None:
                        continue
                    if mid == "snk":
                        nc.vector.tensor_mul(
                            p[0:SINK, bi * 512:(bi + 1) * 512],
                            p[0:SINK, bi * 512:(bi + 1) * 512],
                            msnk[0:SINK, c * 512:(c + 1) * 512],
                        )
                    else:
                        nc.vector.tensor_mul(
                            p[:, bi * 512:(bi + 1) * 512],
                            p[:, bi * 512:(bi + 1) * 512],
                            mp[mid][:],
                        )

        def emit_attn_PV(g, b):
            ysA = psY.tile([65, 512], F32, tag="y", name=f"ysA{g}{b}")
            ysB = psY.tile([65, 512], F32, tag="y", name=f"ysB{g}{b}")
            ys[(g, b)] = (ysA, ysB)
            p = ptile[(g, b)]
            started = [False, False]
            n_blocks = [0, 0]
            for _t, _tki, c, _m in S_BLOCKS:
                n_blocks[c] += 1
            seen = [0, 0]
            for bi, (tag, tki, c, _m) in enumerate(S_BLOCKS):
                dst = ysA if c == 0 else ysB
                seen[c] += 1
                first = not started[c]
                started[c] = True
                last = seen[c] == n_blocks[c]
                if tki is None:
                    lhsT = vs[b][0:SINK, :].rearrange(
                        "p (g c) -> p g c", g=NKV)[:, g:g + 1, :]
                    rhs = p[0:SINK, bi * 512:(bi + 1) * 512]
                else:
                    lhsT = vt[(tki, b)][:].rearrange(
                        "p (g c) -> p g c", g=NKV)[:, g:g + 1, :]
                    rhs = p[:, bi * 512:(bi + 1) * 512]
                nc.tensor.matmul(dst[:], lhsT, rhs, start=first, stop=last)

        def emit_attn_norm(g, b):
            kb = (g % 2) * 64
            m0 = 4 * (g // 2)
            ysA, ysB = ys[(g, b)]
            stg = None
            if kb:
                stg = misc.tile([64, 1024], MM, tag="stg", name=f"stg{g}{b}")
            dn = misc.tile([65, 1024], FR, tag="dn", name=f"dn{g}{b}")
            nc.scalar.copy(dn[64:65, 0:512], ysA[64:65, 0:512])
            nc.scalar.copy(dn[64:65, 512:1024], ysB[64:65, 0:512])
            rbp = psP.tile([128, 1024], F32, tag="s", name=f"rbp{g}{b}")
            nc.tensor.matmul(rbp[0:64, 0:512], onb[64:65, :], dn[64:65, 0:512],
                             start=True, stop=True)
            nc.tensor.matmul(rbp[0:64, 512:1024], onb[64:65, :],
                             dn[64:65, 512:1024], start=True, stop=True)
            rb = misc.tile([64, 1024], F32, tag="rb", name=f"rb{g}{b}")
            nc.vector.reciprocal_approx_fast(rb[:], rbp[0:64, :])
            for ci, ysx in ((0, ysA), (1, ysB)):
                src = ysx[0:64, 0:512].rearrange("p (h q) -> p h q", h=4)
                rbv = rb[:, ci * 512:(ci + 1) * 512].rearrange(
                    "p (h q) -> p h q", h=4)
                if kb == 0:
                    dst = yt[0:64].rearrange(
                        "p (m b q) -> p m b q", m=8, b=B
                    )[:, m0:m0 + 4, b:b + 1, ci * 128:(ci + 1) * 128]
                    nc.vector.tensor_tensor(
                        dst, src, rbv, op=MULT,
                    )
                else:
                    dst = stg[:].rearrange(
                        "p (h q) -> p h q", h=4)[:, :, ci * 128:(ci + 1) * 128]
                    nc.vector.tensor_tensor(
                        dst, src, rbv, op=MULT,
                    )
            if kb:
                nc.sync.dma_start(
                    yt[64:128].rearrange(
                        "p (m b q) -> p m b q", m=8, b=B
                    )[:, m0:m0 + 4, b:b + 1, :],
                    stg[:].rearrange("p (h q) -> p h q", h=4),
                )

        def emit_O(b, mq2, nk, copy_eng="act"):
            po = psP.tile([128, 1024], F32, tag="s", name=f"po{b}{mq2}{nk}")
            for m in range(8):
                nc.tensor.matmul(
                    po[:, 0:512],
                    yt[:, m * (B * QB) + b * QB + mq2 * 128:
                       m * (B * QB) + b * QB + (mq2 + 1) * 128],
                    wo[m][:, nk * 512:(nk + 1) * 512],
                    start=(m == 0), stop=(m == 7),
                )
            ost = misc.tile([128, 512], F32, tag="ost", name=f"o{b}{mq2}{nk}")
            if copy_eng == "act":
                nc.scalar.copy(ost[:], po[:, 0:512])
            else:
                nc.vector.tensor_copy(ost[:], po[:, 0:512])
            nc.sync.dma_start(
                out_d[b, mq2 * 128:(mq2 + 1) * 128, nk * 512:(nk + 1) * 512],
                ost[:],
            )

        # ------------------------------------------------ main schedule
        emit_transposes(0)
        emit_K(0)
        emit_V(0)
        emit_Q(0)

        # Software-pipelined attention: per step emit norm(i-2), S(i),
        # PV(i-1) so the PE never waits on the exp->mask chain.  kb=64
        # groups first so the per-batch tail writes yt directly (no staging
        # DMA on the critical tail).
        GORDER = [1, 3, 0, 2]
        iters = [(g, 0) for g in GORDER] + [(g, 1) for g in GORDER]
        fillers = {0: [lambda: emit_transposes(1)],
                   1: [lambda: emit_K(1)],
                   2: [lambda: emit_V(1)],
                   3: [lambda: emit_Q(1)],
                   5: [lambda: emit_O(0, 0, 0)],
                   6: [lambda: emit_O(0, 0, 1), lambda: emit_O(0, 1, 0)],
                   7: [lambda: emit_O(0, 1, 1)]}
        for i, (g, b) in enumerate(iters):
            if i >= 2:
                emit_attn_norm(*iters[i - 2])
            emit_attn_S(g, b)
            if i >= 1:
                emit_attn_PV(*iters[i - 1])
            for f in fillers.get(i, []):
                f()
        emit_attn_norm(*iters[6])
        emit_attn_PV(*iters[7])
        emit_attn_norm(*iters[7])

        for oi, (mq2, nk) in enumerate([(0, 0), (0, 1), (1, 0), (1, 1)]):
            emit_O(1, mq2, nk, copy_eng="act" if oi % 2 == 0 else "dve")

    nc.compile()
    return nc


# ================================================================ host side
def host_prep(X, Wq, Wk, Wv, Wo):
    """Returns in_maps (list of per-core dicts of numpy arrays)."""
    import ml_dtypes
    bf16 = np.dtype(ml_dtypes.bfloat16)

    X = np.asarray(X, dtype=np.float32)
    Wq = np.asarray(Wq, dtype=np.float32)
    Wk = np.asarray(Wk, dtype=np.float32)
    Wv = np.asarray(Wv, dtype=np.float32)
    Wo = np.asarray(Wo, dtype=np.float32)

    flat_perm = np.concatenate(
        [np.arange(h * HD, (h + 1) * HD) for h in HEAD_ORDER]
    )
    wq_p = (np.ascontiguousarray(Wq[:, flat_perm])
            * np.float32(1.0 / np.sqrt(HD))).astype(bf16)
    wo_p = np.ascontiguousarray(Wo[flat_perm, :]).astype(bf16)
    wk_c = Wk.astype(bf16)
    wv_c = Wv.astype(bf16)

    tt = np.arange(T)
    i = tt[:, None]
    j = tt[None, :]
    m_full = ((j <= i) & ((j < SINK) | (j >= np.maximum(i - WIN + 1, 0)))
              ).astype(np.float32)

    X16 = X.astype(bf16)
    xs = np.ascontiguousarray(X16[:, 0:SINK, :])
    idt = np.eye(128, dtype=np.float32)
    onbm = np.ones((65, 64), dtype=np.float32)

    in_maps = []
    for c in range(NCORES):
        qs = c * QB
        ks = qs - QB

        xw = np.zeros((B, KW, D), dtype=bf16)
        lo = max(ks, 0)
        xw[:, lo - ks:, :] = X16[:, lo:ks + KW, :]

        def wblk(tki, ch):
            jg = ks + tki * 128 + np.arange(128)
            ig = qs + ch * 128 + np.arange(128)
            msk = np.zeros((128, 128), dtype=np.float32)
            vr = jg >= 0
            if vr.any():
                msk[vr, :] = m_full[np.ix_(ig, jg[vr])].T
            return np.tile(msk, (1, 4))

        mpv = np.stack([wblk(0, 0), wblk(1, 1), wblk(2, 0), wblk(3, 1)])

        mts = np.zeros((SINK, QB), dtype=np.float32)
        for jj in range(SINK):
            for ch in range(2):
                cov_lo = ks + ch * 128
                cov_hi = cov_lo + 384
                if cov_lo <= jj < cov_hi:
                    continue  # delivered by window blocks
                mm_ = m_full[qs + ch * 128:qs + (ch + 1) * 128, jj]
                mts[jj, ch * 128:(ch + 1) * 128] = mm_
        msnk = np.concatenate(
            [np.tile(mts[:, 0:128], (1, 4)), np.tile(mts[:, 128:256], (1, 4))],
            axis=1,
        )

        onew = np.zeros((4, 128, NKV), dtype=np.float32)
        for tki in range(4):
            real = (ks + tki * 128 + np.arange(128)) >= 0
            onew[tki, real, :] = 1.0

        in_maps.append({
            "Xw": xw,
            "Xs": xs,
            "IDT": idt.astype(bf16),
            "Wq": wq_p,
            "Wk": wk_c,
            "Wv": wv_c,
            "Wo": wo_p,
            "MP": mpv.astype(bf16),
            "MSNK": msnk.astype(bf16),
            "ONEW": onew.astype(bf16),
            "ONB": onbm,
        })
    return in_maps


_NC_CACHE = {}


def get_nc():
    if "nc" not in _NC_CACHE:
        _NC_CACHE["nc"] = build_nc()
    return _NC_CACHE["nc"]


def kernel(X, Wq, Wk, Wv, Wo):
    in_maps = host_prep(X, Wq, Wk, Wv, Wo)
    nc = get_nc()
    res = run_bass_kernel_spmd(nc, in_maps, list(range(NCORES)))
    out = np.empty((B, T, D), dtype=np.float32)
    for c in range(NCORES):
        out[:, c * QB:(c + 1) * QB, :] = res.results[c]["out"]
    return out


# revision 50
# speedup vs baseline: 1.0428x; 1.0428x over previous
"""AttentionSinkPrefill Trainium2 kernel v2 (8 NeuronCores, sequence-parallel).

Module:   Y = AttnSinkPrefill(X) with sink=4, window=256, causal GQA
          (16 q heads, 4 kv heads, head_dim 64, d_model 1024, B=2, T=2048).

Sharding: sequence-parallel over T.  Core c handles queries
          [256c, 256c+256) for both batches; needs X rows
          [256c-256, 256c+256) (zero-padded left) plus the 4 sink rows.
          No collectives; outputs concatenate.

v2 changes vs v1 (301779ns / 253869ns baseline):
  - bf16 matmul operands everywhere (measured rel err 3.8e-3, gate 2e-2)
  - DMA issue order: X window tiles for batch 0 land first; weights and
    Wo stream behind compute (v1 stalled the PE 34us on weight DMAs)
  - block-sparse attention over 128x128 diagonal blocks: per 256-query
    half only 3 of 4 key blocks are live; the 2 structurally-empty
    blocks are never computed, full blocks skip masking entirely
  - 4 q-heads sharing a kv head are processed per iteration with N=512
    matmuls via strided rhs views into one merged Q tile (4x fewer
    stationary-weight loads in attention)
  - per-core triangular block masks + fine-grained sink mask keep the
    program uniform across cores; zero "ones-columns" kill padding-key
    denominator contributions on boundary cores
  - b1 projections / O-proj are interleaved as PE filler between
    attention iterations so the tensor engine stays continuously busy
    (TRN2 PE only reaches 2.4GHz after 3us without gaps; idle drops it
    to 1.2GHz)
"""

import os
import sys
from contextlib import ExitStack

import numpy as np

sys.path.insert(0, "/opt/trn_rl_repo")

import concourse.bass as bass
import concourse.bacc as bacc
import concourse.mybir as mybir
import concourse.tile as tile
from concourse.bass_utils import run_bass_kernel_spmd

# ---------------------------------------------------------------- constants
D = 1024          # d_model
NH = 16           # q heads
NKV = 4           # kv heads
HD = 64           # head dim
SINK = 4          # attention sink width
WIN = 256         # sliding window
B = 2
T = 2048
NCORES = 8
QB = T // NCORES  # queries per core = 256
KW = 2 * QB       # window key rows per core = 512
KCOL = KW + SINK  # 516 key columns per batch in X^T layout

F32 = mybir.dt.float32
FR = mybir.dt.float32r
MM = mybir.dt.bfloat16
AF = mybir.ActivationFunctionType
MULT = mybir.AluOpType.mult

# head order placing each q head at partition base (kv_head%2)*64, with the
# 4 q heads of kv head g in consecutive m-tiles 4*(g//2)..+4 at that base;
# Wq columns / Wo rows are permuted to match.
HEAD_ORDER = [0, 4, 1, 5, 2, 6, 3, 7, 8, 12, 9, 13, 10, 14, 11, 15]

# attention blocks per (g, b) iteration: (tag, tki, chalf, mask_id)
#   mask_id: 0-3 -> per-core triangular mask tile, 'snk' -> sink mask, None -> full
S_BLOCKS = [
    ("P0", 0, 0, 0),
    ("P1a", 1, 0, None),
    ("P1b", 1, 1, 1),
    ("P2a", 2, 0, 2),
    ("P2b", 2, 1, None),
    ("P3", 3, 1, 3),
    ("S0a", None, 0, "snk"),   # sink keys, c0
    ("S0b", None, 1, "snk"),   # sink keys, c1
]
# PV accumulation target (0 -> ysA/c0, 1 -> ysB/c1) per block follows chalf.


# ================================================================ program
def build_nc():
    nc = bacc.Bacc()

    xw_d = nc.dram_tensor("Xw", [B, KW, D], MM, kind="ExternalInput")
    xs_d = nc.dram_tensor("Xs", [B, SINK, D], MM, kind="ExternalInput")
    idt_d = nc.dram_tensor("IDT", [128, 128], MM, kind="ExternalInput")
    wq_d = nc.dram_tensor("Wq", [D, NH * HD], MM, kind="ExternalInput")
    wk_d = nc.dram_tensor("Wk", [D, NKV * HD], MM, kind="ExternalInput")
    wv_d = nc.dram_tensor("Wv", [D, NKV * HD], MM, kind="ExternalInput")
    wo_d = nc.dram_tensor("Wo", [NH * HD, D], MM, kind="ExternalInput")
    mp_d = nc.dram_tensor("MP", [4, 128, 4 * 128], MM, kind="ExternalInput")
    msnk_d = nc.dram_tensor("MSNK", [SINK, 2 * 512], MM, kind="ExternalInput")
    onew_d = nc.dram_tensor("ONEW", [4, 128, NKV], MM, kind="ExternalInput")
    onb_d = nc.dram_tensor("ONB", [65, 64], FR, kind="ExternalInput")
    out_d = nc.dram_tensor("out", [B, QB, D], F32, kind="ExternalOutput")

    with nc.allow_low_precision(reason="bf16 matmul operands"), \
            tile.TileContext(nc) as tc, ExitStack() as ctx:
        cst = ctx.enter_context(tc.tile_pool(name="cst", bufs=1))
        wts = ctx.enter_context(tc.tile_pool(name="wts", bufs=1))
        wop = ctx.enter_context(tc.tile_pool(name="wop", bufs=1))
        xtp = ctx.enter_context(tc.tile_pool(name="xtp", bufs=1))
        qkv = ctx.enter_context(tc.tile_pool(name="qkv", bufs=1))
        ytp = ctx.enter_context(tc.tile_pool(name="ytp", bufs=1))
        ppool = ctx.enter_context(tc.tile_pool(name="pp", bufs=2))
        misc = ctx.enter_context(tc.tile_pool(name="misc", bufs=2))
        # 3 x [128,1024] (2 banks each) ring shared by projections and the
        # attention S-stream + 2 x [65,512] (1 bank) ys accumulators = 8 banks
        psP = ctx.enter_context(tc.tile_pool(name="psP", bufs=3, space="PSUM"))
        psY = ctx.enter_context(tc.tile_pool(name="psY", bufs=2, space="PSUM"))

        # ------------------------------------------------ persistent tiles
        # X^T per batch: [128, (d-tile, key)], built by all-bf16 PE
        # transposes (1 cycle/row) from bf16 X tiles
        xtb = [xtp.tile([128, 8 * KCOL], MM, tag=f"xtb{b}", name=f"xtb{b}")
               for b in range(B)]

        def xt(b, d):
            return xtb[b][:, d * KCOL:d * KCOL + KCOL]

        # ------------------------------------------------ prologue DMAs
        # X tiles first (the PE's first work), weights stream behind
        ident = cst.tile([128, 128], MM, tag="idt")
        nc.sync.dma_start(ident[:], idt_d[:])
        xws = {}
        for b in range(B):
            for tki in range(4):
                t = xtp.tile([128, D], MM, tag=f"xw{tki}_{b}",
                             name=f"xw{tki}_{b}")
                xws[(b, tki)] = t
            t = xtp.tile([SINK, D], MM, tag=f"xs_{b}", name=f"xs_{b}")
            xws[(b, "s")] = t
        for b in range(B):
            for tki in range(4):
                nc.sync.dma_start(
                    xws[(b, tki)][:], xw_d[b, tki * 128:(tki + 1) * 128, :])
            nc.sync.dma_start(xws[(b, "s")][:], xs_d[b])

        wk = []
        wv = []
        for d in range(8):
            t = wts.tile([128, NKV * HD], MM, tag=f"wk{d}", name=f"wk{d}")
            nc.gpsimd.dma_start(t[:], wk_d[d * 128:(d + 1) * 128, :])
            wk.append(t)
            t = wts.tile([128, NKV * HD], MM, tag=f"wv{d}", name=f"wv{d}")
            nc.gpsimd.dma_start(t[:], wv_d[d * 128:(d + 1) * 128, :])
            wv.append(t)

        wq = []
        for d in range(8):
            t = wts.tile([128, NH * HD], MM, tag=f"wq{d}", name=f"wq{d}")
            nc.gpsimd.dma_start(t[:], wq_d[d * 128:(d + 1) * 128, :])
            wq.append(t)

        mp = []
        for i in range(4):
            t = cst.tile([128, 512], MM, tag=f"mp{i}", name=f"mp{i}")
            nc.gpsimd.dma_start(t[:], mp_d[i])
            mp.append(t)
        msnk = cst.tile([SINK, 1024], MM, tag="msnk")
        nc.gpsimd.dma_start(msnk[:], msnk_d[:])
        onb = cst.tile([65, 64], FR, tag="onb")
        nc.gpsimd.dma_start(onb[:], onb_d[:])

        wo = []
        for m in range(8):
            t = wop.tile([128, D], MM, tag=f"wo{m}", name=f"wo{m}")
            nc.gpsimd.dma_start(t[:], wo_d[m * 128:(m + 1) * 128, :])
            wo.append(t)
        qt = qkv.tile([128, 8 * B * QB], MM, tag="qt")  # [128,(m,b,q)] merged
        kt = [qkv.tile([128, B * KW], MM, tag=f"kt{m}", name=f"kt{m}")
              for m in range(2)]
        ktp = {}
        for mk in range(2):
            for b in range(B):
                t = qkv.tile([128, 128], MM, tag=f"ktp{mk}{b}", name=f"ktp{mk}{b}")
                nc.gpsimd.memset(t[:], 0.0)
                ktp[(mk, b)] = t
        vt = {}
        for tki in range(4):
            for b in range(B):
                t = qkv.tile([128, NKV * (HD + 1)], MM,
                              tag=f"vt{tki}{b}", name=f"vt{tki}{b}")
                nc.sync.dma_start(t[:, HD:NKV * (HD + 1):HD + 1], onew_d[tki])
                vt[(tki, b)] = t
        vs = {}
        for b in range(B):
            t = qkv.tile([SINK, NKV * (HD + 1)], MM, tag=f"vs{b}", name=f"vs{b}")
            nc.gpsimd.memset(t[0:SINK, HD:NKV * (HD + 1):HD + 1], 1.0)
            vs[b] = t
        yt = ytp.tile([128, 8 * B * QB], MM, tag="yt")  # [128,(m,b,q)]

        # ------------------------------------------------ emit helpers
        def emit_transposes(b):
            # tki-outer so the first transpose only needs the first X tile;
            # all-bf16: 1 cycle/row on the PE, 2x-mode copies.  X^T copies
            # go to ACT for b0 (idle prologue) and DVE for b1 (filler time)
            for tki in range(4):
                ps = psP.tile([128, 2048], MM, tag="s", name=f"trp{b}{tki}")
                for d in range(8):
                    nc.tensor.transpose(
                        ps[:, d * 128:(d + 1) * 128],
                        xws[(b, tki)][:, d * 128:(d + 1) * 128],
                        ident[:],
                    )
                for d in range(8):
                    dst = xt(b, d)[:, tki * 128:(tki + 1) * 128]
                    src = ps[:, d * 128:(d + 1) * 128]
                    if b == 0:
                        nc.scalar.copy(dst, src)
                    else:
                        nc.vector.tensor_copy(dst, src)
            # sink rows: 8 transposes [128,4] packed into one psum tile
            ps = psP.tile([128, 2048], MM, tag="s", name=f"trs{b}")
            for d in range(8):
                nc.tensor.transpose(
                    ps[:, d * 4:d * 4 + SINK],
                    xws[(b, "s")][0:SINK, d * 128:(d + 1) * 128],
                    ident[0:SINK, 0:SINK],
                )
            for d in range(8):
                nc.vector.tensor_copy(
                    xt(b, d)[:, KW:KCOL],
                    ps[:, d * 4:d * 4 + SINK],
                )

        def emit_K(b):
            for mk in range(2):
                ps = psP.tile([128, 1024], F32, tag="s", name=f"kps{b}{mk}")
                for d in range(8):
                    nc.tensor.matmul(
                        ps[:, 0:KW],
                        wk[d][:, mk * 128:(mk + 1) * 128],
                        xt(b, d)[:, 0:KW],
                        start=(d == 0), stop=(d == 7),
                    )
                for d in range(8):
                    nc.tensor.matmul(
                        ps[:, KW:KW + SINK],
                        wk[d][:, mk * 128:(mk + 1) * 128],
                        xt(b, d)[:, KW:KCOL],
                        start=(d == 0), stop=(d == 7),
                    )
                nc.vector.tensor_copy(
                    kt[mk][:, b * KW:(b + 1) * KW], ps[:, 0:KW])
                nc.vector.tensor_copy(ktp[(mk, b)][:, 0:SINK], ps[:, KW:KW + SINK])

        def emit_V(b):
            for tki in range(4):
                ps = psP.tile([128, 1024], F32, tag="s", name=f"vps{b}{tki}")
                for d in range(8):
                    nc.tensor.matmul(
                        ps[:, 0:NKV * HD],
                        xt(b, d)[:, tki * 128:(tki + 1) * 128],
                        wv[d][:],
                        start=(d == 0), stop=(d == 7),
                    )
                nc.vector.tensor_copy(
                    vt[(tki, b)][:].rearrange(
                        "p (g c) -> p g c", g=NKV)[:, :, 0:HD],
                    ps[:, 0:NKV * HD].rearrange("p (g c) -> p g c", g=NKV),
                )
            ps = psP.tile([128, 1024], F32, tag="s", name=f"vsps{b}")
            for d in range(8):
                nc.tensor.matmul(
                    ps[0:SINK, 0:NKV * HD],
                    xt(b, d)[:, KW:KCOL],
                    wv[d][:],
                    start=(d == 0), stop=(d == 7),
                )
            nc.vector.tensor_copy(
                vs[b][0:SINK].rearrange("p (g c) -> p g c", g=NKV)[:, :, 0:HD],
                ps[0:SINK, 0:NKV * HD].rearrange("p (g c) -> p g c", g=NKV),
            )

        def emit_Q(b):
            # two head-tiles per psum tile, one strided copy into qt
            for j in range(4):
                ps = psP.tile([128, 1024], F32, tag="s", name=f"qps{b}{j}")
                for mi in range(2):
                    m = 2 * j + mi
                    for d in range(8):
                        nc.tensor.matmul(
                            ps[:, mi * QB:(mi + 1) * QB],
                            wq[d][:, m * 128:(m + 1) * 128],
                            xt(b, d)[:, KW - QB:KW],
                            start=(d == 0), stop=(d == 7),
                        )
                nc.vector.tensor_copy(
                    qt[:].rearrange("p (m b q) -> p m b q", m=8, b=B)
                      [:, 2 * j:2 * j + 2, b:b + 1, :],
                    ps[:, 0:2 * QB].rearrange("p (m b q) -> p m b q", m=2, b=1),
                )

        # attention state carried across emit calls
        sps = {}    # (g,b) -> list of S psum tiles (same order as S_BLOCKS)
        ys = {}     # (g,b) -> (ysA, ysB)
        ptile = {}  # (g,b) -> p tile

        def qview(g, b, c):
            kb = (g % 2) * 64
            m0 = 4 * (g // 2)
            return qt[kb:kb + 64].rearrange(
                "p (m b q) -> p m b q", m=8, b=B
            )[:, m0:m0 + 4, b:b + 1, c * 128:(c + 1) * 128]

        def emit_attn_S(g, b):
            # 8 S matmuls into 4 two-bank psum tiles; one exp per pair
            kb = (g % 2) * 64
            mk = g // 2
            p = ppool.tile([128, 8 * 512], MM, tag="p", name=f"p{g}{b}")
            ptile[(g, b)] = p
            for pair in range(4):
                sp = psP.tile([128, 1024], F32, tag="s", name=f"s{g}{b}{pair}")
                for half in range(2):
                    bi = pair * 2 + half
                    tag, tki, c, _m = S_BLOCKS[bi]
                    if tki is None:
                        lhsT = ktp[(mk, b)][kb:kb + 64, :]
                    else:
                        lhsT = kt[mk][kb:kb + 64,
                                      b * KW + tki * 128:b * KW + (tki + 1) * 128]
                    nc.tensor.matmul(sp[:, half * 512:(half + 1) * 512],
                                     lhsT, qview(g, b, c),
                                     start=True, stop=True)
                nc.scalar.activation(
                    p[:, pair * 1024:(pair + 1) * 1024], sp[:], AF.Exp)
                for half in range(2):
                    bi = pair * 2 + half
                    tag, tki, c, mid = S_BLOCKS[bi]
                    if mid is None:
                        continue
                    if mid == "snk":
                        nc.vector.tensor_mul(
                            p[0:SINK, bi * 512:(bi + 1) * 512],
                            p[0:SINK, bi * 512:(bi + 1) * 512],
                            msnk[0:SINK, c * 512:(c + 1) * 512],
                        )
                    else:
                        nc.vector.tensor_mul(
                            p[:, bi * 512:(bi + 1) * 512],
                            p[:, bi * 512:(bi + 1) * 512],
                            mp[mid][:],
                        )

        def emit_attn_PV(g, b):
            ysA = psY.tile([65, 512], F32, tag="y", name=f"ysA{g}{b}")
            ysB = psY.tile([65, 512], F32, tag="y", name=f"ysB{g}{b}")
            ys[(g, b)] = (ysA, ysB)
            p = ptile[(g, b)]
            started = [False, False]
            n_blocks = [0, 0]
            for _t, _tki, c, _m in S_BLOCKS:
                n_blocks[c] += 1
            seen = [0, 0]
            for bi, (tag, tki, c, _m) in enumerate(S_BLOCKS):
                dst = ysA if c == 0 else ysB
                seen[c] += 1
                first = not started[c]
                started[c] = True
                last = seen[c] == n_blocks[c]
                if tki is None:
                    lhsT = vs[b][0:SINK, :].rearrange(
                        "p (g c) -> p g c", g=NKV)[:, g:g + 1, :]
                    rhs = p[0:SINK, bi * 512:(bi + 1) * 512]
                else:
                    lhsT = vt[(tki, b)][:].rearrange(
                        "p (g c) -> p g c", g=NKV)[:, g:g + 1, :]
                    rhs = p[:, bi * 512:(bi + 1) * 512]
                nc.tensor.matmul(dst[:], lhsT, rhs, start=first, stop=last)

        def emit_attn_norm(g, b):
            kb = (g % 2) * 64
            m0 = 4 * (g // 2)
            ysA, ysB = ys[(g, b)]
            stg = None
            if kb:
                stg = misc.tile([64, 1024], MM, tag="stg", name=f"stg{g}{b}")
            dn = misc.tile([65, 1024], FR, tag="dn", name=f"dn{g}{b}")
            nc.scalar.copy(dn[64:65, 0:512], ysA[64:65, 0:512])
            nc.scalar.copy(dn[64:65, 512:1024], ysB[64:65, 0:512])
            rbp = psP.tile([128, 1024], F32, tag="s", name=f"rbp{g}{b}")
            nc.tensor.matmul(rbp[0:64, 0:512], onb[64:65, :], dn[64:65, 0:512],
                             start=True, stop=True)
            nc.tensor.matmul(rbp[0:64, 512:1024], onb[64:65, :],
                             dn[64:65, 512:1024], start=True, stop=True)
            rb = misc.tile([64, 1024], F32, tag="rb", name=f"rb{g}{b}")
            nc.vector.reciprocal_approx_fast(rb[:], rbp[0:64, :])
            for ci, ysx in ((0, ysA), (1, ysB)):
                src = ysx[0:64, 0:512].rearrange("p (h q) -> p h q", h=4)
                rbv = rb[:, ci * 512:(ci + 1) * 512].rearrange(
                    "p (h q) -> p h q", h=4)
                if kb == 0:
                    dst = yt[0:64].rearrange(
                        "p (m b q) -> p m b q", m=8, b=B
                    )[:, m0:m0 + 4, b:b + 1, ci * 128:(ci + 1) * 128]
                    nc.vector.tensor_tensor(
                        dst, src, rbv, op=MULT,
                    )
                else:
                    dst = stg[:].rearrange(
                        "p (h q) -> p h q", h=4)[:, :, ci * 128:(ci + 1) * 128]
                    nc.vector.tensor_tensor(
                        dst, src, rbv, op=MULT,
                    )
            if kb:
                nc.sync.dma_start(
                    yt[64:128].rearrange(
                        "p (m b q) -> p m b q", m=8, b=B
                    )[:, m0:m0 + 4, b:b + 1, :],
                    stg[:].rearrange("p (h q) -> p h q", h=4),
                )

        def emit_O(b, mq2, nk, copy_eng="act"):
            po = psP.tile([128, 1024], F32, tag="s", name=f"po{b}{mq2}{nk}")
            for m in range(8):
                nc.tensor.matmul(
                    po[:, 0:512],
                    yt[:, m * (B * QB) + b * QB + mq2 * 128:
                       m * (B * QB) + b * QB + (mq2 + 1) * 128],
                    wo[m][:, nk * 512:(nk + 1) * 512],
                    start=(m == 0), stop=(m == 7),
                )
            ost = misc.tile([128, 512], F32, tag="ost", name=f"o{b}{mq2}{nk}")
            if copy_eng == "act":
                nc.scalar.copy(ost[:], po[:, 0:512])
            else:
                nc.vector.tensor_copy(ost[:], po[:, 0:512])
            nc.sync.dma_start(
                out_d[b, mq2 * 128:(mq2 + 1) * 128, nk * 512:(nk + 1) * 512],
                ost[:],
            )

        # ------------------------------------------------ main schedule
        emit_transposes(0)
        emit_K(0)
        emit_V(0)
        emit_Q(0)

        # Software-pipelined attention: per step emit norm(i-2), S(i),
        # PV(i-1) so the PE never waits on the exp->mask chain.  kb=64
        # groups first so the per-batch tail writes yt directly (no staging
        # DMA on the critical tail).
        GORDER = [1, 3, 0, 2]
        iters = [(g, 0) for g in GORDER] + [(g, 1) for g in GORDER]
        fillers = {0: [lambda: emit_transposes(1)],
                   1: [lambda: emit_K(1)],
                   2: [lambda: emit_V(1)],
                   3: [lambda: emit_Q(1)],
                   5: [lambda: emit_O(0, 0, 0)],
                   6: [lambda: emit_O(0, 0, 1), lambda: emit_O(0, 1, 0)],
                   7: [lambda: emit_O(0, 1, 1)]}
        for i, (g, b) in enumerate(iters):
            if i >= 2:
                emit_attn_norm(*iters[i - 2])
            emit_attn_S(g, b)
            if i >= 1:
                emit_attn_PV(*iters[i - 1])
            for f in fillers.get(i, []):
                f()
        emit_attn_norm(*iters[6])
        emit_attn_PV(*iters[7])
        emit_attn_norm(*iters[7])

        for oi, (mq2, nk) in enumerate([(0, 0), (0, 1), (1, 0), (1, 1)]):
            emit_O(1, mq2, nk, copy_eng="act" if oi % 2 == 0 else "dve")

    nc.compile()
    return nc


# ================================================================ host side
def host_prep(X, Wq, Wk, Wv, Wo):
    """Returns in_maps (list of per-core dicts of numpy arrays)."""
    import ml_dtypes
    bf16 = np.dtype(ml_dtypes.bfloat16)

    X = np.asarray(X, dtype=np.float32)
    Wq = np.asarray(Wq, dtype=np.float32)
    Wk = np.asarray(Wk, dtype=np.float32)
    Wv = np.asarray(Wv, dtype=np.float32)
    Wo = np.asarray(Wo, dtype=np.float32)

    flat_perm = np.concatenate(
        [np.arange(h * HD, (h + 1) * HD) for h in HEAD_ORDER]
    )
    wq_p = (np.ascontiguousarray(Wq[:, flat_perm])
            * np.float32(1.0 / np.sqrt(HD))).astype(bf16)
    wo_p = np.ascontiguousarray(Wo[flat_perm, :]).astype(bf16)
    wk_c = Wk.astype(bf16)
    wv_c = Wv.astype(bf16)

    tt = np.arange(T)
    i = tt[:, None]
    j = tt[None, :]
    m_full = ((j <= i) & ((j < SINK) | (j >= np.maximum(i - WIN + 1, 0)))
              ).astype(np.float32)

    X16 = X.astype(bf16)
    xs = np.ascontiguousarray(X16[:, 0:SINK, :])
    idt = np.eye(128, dtype=np.float32)
    onbm = np.ones((65, 64), dtype=np.float32)

    in_maps = []
    for c in range(NCORES):
        qs = c * QB
        ks = qs - QB

        xw = np.zeros((B, KW, D), dtype=bf16)
        lo = max(ks, 0)
        xw[:, lo - ks:, :] = X16[:, lo:ks + KW, :]

        def wblk(tki, ch):
            jg = ks + tki * 128 + np.arange(128)
            ig = qs + ch * 128 + np.arange(128)
            msk = np.zeros((128, 128), dtype=np.float32)
            vr = jg >= 0
            if vr.any():
                msk[vr, :] = m_full[np.ix_(ig, jg[vr])].T
            return np.tile(msk, (1, 4))

        mpv = np.stack([wblk(0, 0), wblk(1, 1), wblk(2, 0), wblk(3, 1)])

        mts = np.zeros((SINK, QB), dtype=np.float32)
        for jj in range(SINK):
            for ch in range(2):
                cov_lo = ks + ch * 128
                cov_hi = cov_lo + 384
                if cov_lo <= jj < cov_hi:
                    continue  # delivered by window blocks
                mm_ = m_full[qs + ch * 128:qs + (ch + 1) * 128, jj]
                mts[jj, ch * 128:(ch + 1) * 128] = mm_
        msnk = np.concatenate(
            [np.tile(mts[:, 0:128], (1, 4)), np.tile(mts[:, 128:256], (1, 4))],
            axis=1,
        )

        onew = np.zeros((4, 128, NKV), dtype=np.float32)
        for tki in range(4):
            real = (ks + tki * 128 + np.arange(128)) >= 0
            onew[tki, real, :] = 1.0

        in_maps.append({
            "Xw": xw,
            "Xs": xs,
            "IDT": idt.astype(bf16),
            "Wq": wq_p,
            "Wk": wk_c,
            "Wv": wv_c,
            "Wo": wo_p,
            "MP": mpv.astype(bf16),
            "MSNK": msnk.astype(bf16),
            "ONEW": onew.astype(bf16),
            "ONB": onbm,
        })
    return in_maps


_NC_CACHE = {}


def get_nc():
    if "nc" not in _NC_CACHE:
        _NC_CACHE["nc"] = build_nc()
    return _NC_CACHE["nc"]


def kernel(X, Wq, Wk, Wv, Wo):
    in_maps = host_prep(X, Wq, Wk, Wv, Wo)
    nc = get_nc()
    res = run_bass_kernel_spmd(nc, in_maps, list(range(NCORES)))
    out = np.empty((B, T, D), dtype=np.float32)
    for c in range(NCORES):
        out[:, c * QB:(c + 1) * QB, :] = res.results[c]["out"]
    return out


# revision 51
# speedup vs baseline: 1.0784x; 1.0342x over previous
"""AttentionSinkPrefill Trainium2 kernel v2 (8 NeuronCores, sequence-parallel).

Module:   Y = AttnSinkPrefill(X) with sink=4, window=256, causal GQA
          (16 q heads, 4 kv heads, head_dim 64, d_model 1024, B=2, T=2048).

Sharding: sequence-parallel over T.  Core c handles queries
          [256c, 256c+256) for both batches; needs X rows
          [256c-256, 256c+256) (zero-padded left) plus the 4 sink rows.
          No collectives; outputs concatenate.

v2 changes vs v1 (301779ns / 253869ns baseline):
  - bf16 matmul operands everywhere (measured rel err 3.8e-3, gate 2e-2)
  - DMA issue order: X window tiles for batch 0 land first; weights and
    Wo stream behind compute (v1 stalled the PE 34us on weight DMAs)
  - block-sparse attention over 128x128 diagonal blocks: per 256-query
    half only 3 of 4 key blocks are live; the 2 structurally-empty
    blocks are never computed, full blocks skip masking entirely
  - 4 q-heads sharing a kv head are processed per iteration with N=512
    matmuls via strided rhs views into one merged Q tile (4x fewer
    stationary-weight loads in attention)
  - per-core triangular block masks + fine-grained sink mask keep the
    program uniform across cores; zero "ones-columns" kill padding-key
    denominator contributions on boundary cores
  - b1 projections / O-proj are interleaved as PE filler between
    attention iterations so the tensor engine stays continuously busy
    (TRN2 PE only reaches 2.4GHz after 3us without gaps; idle drops it
    to 1.2GHz)
"""

import os
import sys
from contextlib import ExitStack

import numpy as np

sys.path.insert(0, "/opt/trn_rl_repo")

import concourse.bass as bass
import concourse.bacc as bacc
import concourse.mybir as mybir
import concourse.tile as tile
from concourse.bass_utils import run_bass_kernel_spmd

# ---------------------------------------------------------------- constants
D = 1024          # d_model
NH = 16           # q heads
NKV = 4           # kv heads
HD = 64           # head dim
SINK = 4          # attention sink width
WIN = 256         # sliding window
B = 2
T = 2048
NCORES = 8
QB = T // NCORES  # queries per core = 256
KW = 2 * QB       # window key rows per core = 512
KCOL = KW + SINK  # 516 key columns per batch in X^T layout

F32 = mybir.dt.float32
FR = mybir.dt.float32r
MM = mybir.dt.bfloat16
AF = mybir.ActivationFunctionType
MULT = mybir.AluOpType.mult

# head order placing each q head at partition base (kv_head%2)*64, with the
# 4 q heads of kv head g in consecutive m-tiles 4*(g//2)..+4 at that base;
# Wq columns / Wo rows are permuted to match.
HEAD_ORDER = [0, 4, 1, 5, 2, 6, 3, 7, 8, 12, 9, 13, 10, 14, 11, 15]

# attention blocks per (g, b) iteration: (tag, tki, chalf, mask_id)
#   mask_id: 0-3 -> per-core triangular mask tile, 'snk' -> sink mask, None -> full
S_BLOCKS = [
    ("P0", 0, 0, 0),
    ("P1a", 1, 0, None),
    ("P1b", 1, 1, 1),
    ("P2a", 2, 0, 2),
    ("P2b", 2, 1, None),
    ("P3", 3, 1, 3),
    ("S0a", None, 0, "snk"),   # sink keys, c0
    ("S0b", None, 1, "snk"),   # sink keys, c1
]
# PV accumulation target (0 -> ysA/c0, 1 -> ysB/c1) per block follows chalf.


# ================================================================ program
def build_nc():
    nc = bacc.Bacc()

    xw_d = nc.dram_tensor("Xw", [B, KW, D], MM, kind="ExternalInput")
    xs_d = nc.dram_tensor("Xs", [B, SINK, D], MM, kind="ExternalInput")
    idt_d = nc.dram_tensor("IDT", [128, 128], MM, kind="ExternalInput")
    wq_d = nc.dram_tensor("Wq", [D, NH * HD], MM, kind="ExternalInput")
    wk_d = nc.dram_tensor("Wk", [D, NKV * HD], MM, kind="ExternalInput")
    wv_d = nc.dram_tensor("Wv", [D, NKV * HD], MM, kind="ExternalInput")
    wo_d = nc.dram_tensor("Wo", [NH * HD, D], MM, kind="ExternalInput")
    mp_d = nc.dram_tensor("MP", [4, 128, 4 * 128], MM, kind="ExternalInput")
    msnk_d = nc.dram_tensor("MSNK", [SINK, 2 * 512], MM, kind="ExternalInput")
    onew_d = nc.dram_tensor("ONEW", [4, 128, NKV], MM, kind="ExternalInput")
    onb_d = nc.dram_tensor("ONB", [65, 64], FR, kind="ExternalInput")
    out_d = nc.dram_tensor("out", [B, QB, D], F32, kind="ExternalOutput")

    with nc.allow_low_precision(reason="bf16 matmul operands"), \
            tile.TileContext(nc) as tc, ExitStack() as ctx:
        cst = ctx.enter_context(tc.tile_pool(name="cst", bufs=1))
        wts = ctx.enter_context(tc.tile_pool(name="wts", bufs=1))
        wop = ctx.enter_context(tc.tile_pool(name="wop", bufs=1))
        xtp = ctx.enter_context(tc.tile_pool(name="xtp", bufs=1))
        qkv = ctx.enter_context(tc.tile_pool(name="qkv", bufs=1))
        ytp = ctx.enter_context(tc.tile_pool(name="ytp", bufs=1))
        ppool = ctx.enter_context(tc.tile_pool(name="pp", bufs=2))
        misc = ctx.enter_context(tc.tile_pool(name="misc", bufs=2))
        # 3 x [128,1024] (2 banks each) ring shared by projections and the
        # attention S-stream + 2 x [65,512] (1 bank) ys accumulators = 8 banks
        psP = ctx.enter_context(tc.tile_pool(name="psP", bufs=3, space="PSUM"))
        psY = ctx.enter_context(tc.tile_pool(name="psY", bufs=2, space="PSUM"))

        # ------------------------------------------------ persistent tiles
        # X^T per batch: [128, (d-tile, key)], built by all-bf16 PE
        # transposes (1 cycle/row) from bf16 X tiles
        xtb = [xtp.tile([128, 8 * KCOL], MM, tag=f"xtb{b}", name=f"xtb{b}")
               for b in range(B)]

        def xt(b, d):
            return xtb[b][:, d * KCOL:d * KCOL + KCOL]

        # ------------------------------------------------ prologue DMAs
        # X tiles first (the PE's first work), weights stream behind
        ident = cst.tile([128, 128], MM, tag="idt")
        nc.sync.dma_start(ident[:], idt_d[:])
        xws = {}
        for b in range(B):
            for tki in range(4):
                t = xtp.tile([128, D], MM, tag=f"xw{tki}_{b}",
                             name=f"xw{tki}_{b}")
                xws[(b, tki)] = t
            t = xtp.tile([SINK, D], MM, tag=f"xs_{b}", name=f"xs_{b}")
            xws[(b, "s")] = t
        for tki in range(4):
            nc.sync.dma_start(
                xws[(0, tki)][:], xw_d[0, tki * 128:(tki + 1) * 128, :])
        nc.sync.dma_start(xws[(0, "s")][:], xs_d[0])

        # wk/wv on sync right behind xw b0: K(0) is the PE's second job
        wk = []
        wv = []
        for d in range(8):
            t = wts.tile([128, NKV * HD], MM, tag=f"wk{d}", name=f"wk{d}")
            nc.sync.dma_start(t[:], wk_d[d * 128:(d + 1) * 128, :])
            wk.append(t)
            t = wts.tile([128, NKV * HD], MM, tag=f"wv{d}", name=f"wv{d}")
            nc.sync.dma_start(t[:], wv_d[d * 128:(d + 1) * 128, :])
            wv.append(t)

        for tki in range(4):
            nc.sync.dma_start(
                xws[(1, tki)][:], xw_d[1, tki * 128:(tki + 1) * 128, :])
        nc.sync.dma_start(xws[(1, "s")][:], xs_d[1])

        mp = []
        for i in range(4):
            t = cst.tile([128, 512], MM, tag=f"mp{i}", name=f"mp{i}")
            nc.gpsimd.dma_start(t[:], mp_d[i])
            mp.append(t)
        msnk = cst.tile([SINK, 1024], MM, tag="msnk")
        nc.gpsimd.dma_start(msnk[:], msnk_d[:])
        onb = cst.tile([65, 64], FR, tag="onb")
        nc.gpsimd.dma_start(onb[:], onb_d[:])

        wq = []
        for d in range(8):
            t = wts.tile([128, NH * HD], MM, tag=f"wq{d}", name=f"wq{d}")
            nc.gpsimd.dma_start(t[:], wq_d[d * 128:(d + 1) * 128, :])
            wq.append(t)

        wo = []
        for m in range(8):
            t = wop.tile([128, D], MM, tag=f"wo{m}", name=f"wo{m}")
            nc.gpsimd.dma_start(t[:], wo_d[m * 128:(m + 1) * 128, :])
            wo.append(t)
        qt = qkv.tile([128, 8 * B * QB], MM, tag="qt")  # [128,(m,b,q)] merged
        kt = [qkv.tile([128, B * KW], MM, tag=f"kt{m}", name=f"kt{m}")
              for m in range(2)]
        ktp = {}
        for mk in range(2):
            for b in range(B):
                t = qkv.tile([128, 128], MM, tag=f"ktp{mk}{b}", name=f"ktp{mk}{b}")
                nc.gpsimd.memset(t[:], 0.0)
                ktp[(mk, b)] = t
        vt = {}
        for tki in range(4):
            for b in range(B):
                t = qkv.tile([128, NKV * (HD + 1)], MM,
                              tag=f"vt{tki}{b}", name=f"vt{tki}{b}")
                nc.sync.dma_start(t[:, HD:NKV * (HD + 1):HD + 1], onew_d[tki])
                vt[(tki, b)] = t
        vs = {}
        for b in range(B):
            t = qkv.tile([SINK, NKV * (HD + 1)], MM, tag=f"vs{b}", name=f"vs{b}")
            nc.gpsimd.memset(t[0:SINK, HD:NKV * (HD + 1):HD + 1], 1.0)
            vs[b] = t
        yt = ytp.tile([128, 8 * B * QB], MM, tag="yt")  # [128,(m,b,q)]

        # ------------------------------------------------ emit helpers
        def emit_transposes(b):
            # tki-outer so the first transpose only needs the first X tile;
            # all-bf16: 1 cycle/row on the PE, 2x-mode copies.  X^T copies
            # go to ACT for b0 (idle prologue) and DVE for b1 (filler time)
            for tki in range(4):
                ps = psP.tile([128, 2048], MM, tag="s", name=f"trp{b}{tki}")
                for d in range(8):
                    nc.tensor.transpose(
                        ps[:, d * 128:(d + 1) * 128],
                        xws[(b, tki)][:, d * 128:(d + 1) * 128],
                        ident[:],
                    )
                for d in range(8):
                    dst = xt(b, d)[:, tki * 128:(tki + 1) * 128]
                    src = ps[:, d * 128:(d + 1) * 128]
                    if b == 0:
                        nc.scalar.copy(dst, src)
                    else:
                        nc.vector.tensor_copy(dst, src)
            # sink rows: 8 transposes [128,4] packed into one psum tile
            ps = psP.tile([128, 2048], MM, tag="s", name=f"trs{b}")
            for d in range(8):
                nc.tensor.transpose(
                    ps[:, d * 4:d * 4 + SINK],
                    xws[(b, "s")][0:SINK, d * 128:(d + 1) * 128],
                    ident[0:SINK, 0:SINK],
                )
            for d in range(8):
                nc.vector.tensor_copy(
                    xt(b, d)[:, KW:KCOL],
                    ps[:, d * 4:d * 4 + SINK],
                )

        def emit_K(b):
            for mk in range(2):
                ps = psP.tile([128, 1024], F32, tag="s", name=f"kps{b}{mk}")
                for d in range(8):
                    nc.tensor.matmul(
                        ps[:, 0:KW],
                        wk[d][:, mk * 128:(mk + 1) * 128],
                        xt(b, d)[:, 0:KW],
                        start=(d == 0), stop=(d == 7),
                    )
                for d in range(8):
                    nc.tensor.matmul(
                        ps[:, KW:KW + SINK],
                        wk[d][:, mk * 128:(mk + 1) * 128],
                        xt(b, d)[:, KW:KCOL],
                        start=(d == 0), stop=(d == 7),
                    )
                nc.vector.tensor_copy(
                    kt[mk][:, b * KW:(b + 1) * KW], ps[:, 0:KW])
                nc.vector.tensor_copy(ktp[(mk, b)][:, 0:SINK], ps[:, KW:KW + SINK])

        def emit_V(b, tkis=(0, 1, 2, 3), sink=True):
            for tki in tkis:
                ps = psP.tile([128, 1024], F32, tag="s", name=f"vps{b}{tki}")
                for d in range(8):
                    nc.tensor.matmul(
                        ps[:, 0:NKV * HD],
                        xt(b, d)[:, tki * 128:(tki + 1) * 128],
                        wv[d][:],
                        start=(d == 0), stop=(d == 7),
                    )
                nc.vector.tensor_copy(
                    vt[(tki, b)][:].rearrange(
                        "p (g c) -> p g c", g=NKV)[:, :, 0:HD],
                    ps[:, 0:NKV * HD].rearrange("p (g c) -> p g c", g=NKV),
                )
            if not sink:
                return
            ps = psP.tile([128, 1024], F32, tag="s", name=f"vsps{b}")
            for d in range(8):
                nc.tensor.matmul(
                    ps[0:SINK, 0:NKV * HD],
                    xt(b, d)[:, KW:KCOL],
                    wv[d][:],
                    start=(d == 0), stop=(d == 7),
                )
            nc.vector.tensor_copy(
                vs[b][0:SINK].rearrange("p (g c) -> p g c", g=NKV)[:, :, 0:HD],
                ps[0:SINK, 0:NKV * HD].rearrange("p (g c) -> p g c", g=NKV),
            )

        def emit_Q(b):
            # two head-tiles per psum tile, one strided copy into qt
            for j in range(4):
                ps = psP.tile([128, 1024], F32, tag="s", name=f"qps{b}{j}")
                for mi in range(2):
                    m = 2 * j + mi
                    for d in range(8):
                        nc.tensor.matmul(
                            ps[:, mi * QB:(mi + 1) * QB],
                            wq[d][:, m * 128:(m + 1) * 128],
                            xt(b, d)[:, KW - QB:KW],
                            start=(d == 0), stop=(d == 7),
                        )
                nc.vector.tensor_copy(
                    qt[:].rearrange("p (m b q) -> p m b q", m=8, b=B)
                      [:, 2 * j:2 * j + 2, b:b + 1, :],
                    ps[:, 0:2 * QB].rearrange("p (m b q) -> p m b q", m=2, b=1),
                )

        # attention state carried across emit calls
        sps = {}    # (g,b) -> list of S psum tiles (same order as S_BLOCKS)
        ys = {}     # (g,b) -> (ysA, ysB)
        ptile = {}  # (g,b) -> p tile

        def qview(g, b, c):
            kb = (g % 2) * 64
            m0 = 4 * (g // 2)
            return qt[kb:kb + 64].rearrange(
                "p (m b q) -> p m b q", m=8, b=B
            )[:, m0:m0 + 4, b:b + 1, c * 128:(c + 1) * 128]

        def emit_attn_S(g, b):
            # 8 S matmuls into 4 two-bank psum tiles; one exp per pair
            kb = (g % 2) * 64
            mk = g // 2
            p = ppool.tile([128, 8 * 512], MM, tag="p", name=f"p{g}{b}")
            ptile[(g, b)] = p
            for pair in range(4):
                sp = psP.tile([128, 1024], F32, tag="s", name=f"s{g}{b}{pair}")
                for half in range(2):
                    bi = pair * 2 + half
                    tag, tki, c, _m = S_BLOCKS[bi]
                    if tki is None:
                        lhsT = ktp[(mk, b)][kb:kb + 64, :]
                    else:
                        lhsT = kt[mk][kb:kb + 64,
                                      b * KW + tki * 128:b * KW + (tki + 1) * 128]
                    nc.tensor.matmul(sp[:, half * 512:(half + 1) * 512],
                                     lhsT, qview(g, b, c),
                                     start=True, stop=True)
                nc.scalar.activation(
                    p[:, pair * 1024:(pair + 1) * 1024], sp[:], AF.Exp)
                for half in range(2):
                    bi = pair * 2 + half
                    tag, tki, c, mid = S_BLOCKS[bi]
                    if mid is None:
                        continue
                    if mid == "snk":
                        nc.vector.tensor_mul(
                            p[0:SINK, bi * 512:(bi + 1) * 512],
                            p[0:SINK, bi * 512:(bi + 1) * 512],
                            msnk[0:SINK, c * 512:(c + 1) * 512],
                        )
                    else:
                        nc.vector.tensor_mul(
                            p[:, bi * 512:(bi + 1) * 512],
                            p[:, bi * 512:(bi + 1) * 512],
                            mp[mid][:],
                        )

        def emit_attn_PV(g, b):
            ysA = psY.tile([65, 512], F32, tag="y", name=f"ysA{g}{b}")
            ysB = psY.tile([65, 512], F32, tag="y", name=f"ysB{g}{b}")
            ys[(g, b)] = (ysA, ysB)
            p = ptile[(g, b)]
            started = [False, False]
            n_blocks = [0, 0]
            for _t, _tki, c, _m in S_BLOCKS:
                n_blocks[c] += 1
            seen = [0, 0]
            for bi, (tag, tki, c, _m) in enumerate(S_BLOCKS):
                dst = ysA if c == 0 else ysB
                seen[c] += 1
                first = not started[c]
                started[c] = True
                last = seen[c] == n_blocks[c]
                if tki is None:
                    lhsT = vs[b][0:SINK, :].rearrange(
                        "p (g c) -> p g c", g=NKV)[:, g:g + 1, :]
                    rhs = p[0:SINK, bi * 512:(bi + 1) * 512]
                else:
                    lhsT = vt[(tki, b)][:].rearrange(
                        "p (g c) -> p g c", g=NKV)[:, g:g + 1, :]
                    rhs = p[:, bi * 512:(bi + 1) * 512]
                nc.tensor.matmul(dst[:], lhsT, rhs, start=first, stop=last)

        def emit_attn_norm(g, b):
            kb = (g % 2) * 64
            m0 = 4 * (g // 2)
            ysA, ysB = ys[(g, b)]
            stg = None
            if kb:
                stg = misc.tile([64, 1024], MM, tag="stg", name=f"stg{g}{b}")
            dn = misc.tile([65, 1024], FR, tag="dn", name=f"dn{g}{b}")
            nc.scalar.copy(dn[64:65, 0:512], ysA[64:65, 0:512])
            nc.scalar.copy(dn[64:65, 512:1024], ysB[64:65, 0:512])
            rbp = psP.tile([128, 1024], F32, tag="s", name=f"rbp{g}{b}")
            nc.tensor.matmul(rbp[0:64, 0:512], onb[64:65, :], dn[64:65, 0:512],
                             start=True, stop=True)
            nc.tensor.matmul(rbp[0:64, 512:1024], onb[64:65, :],
                             dn[64:65, 512:1024], start=True, stop=True)
            rb = misc.tile([64, 1024], F32, tag="rb", name=f"rb{g}{b}")
            nc.vector.reciprocal_approx_fast(rb[:], rbp[0:64, :])
            for ci, ysx in ((0, ysA), (1, ysB)):
                src = ysx[0:64, 0:512].rearrange("p (h q) -> p h q", h=4)
                rbv = rb[:, ci * 512:(ci + 1) * 512].rearrange(
                    "p (h q) -> p h q", h=4)
                if kb == 0:
                    dst = yt[0:64].rearrange(
                        "p (m b q) -> p m b q", m=8, b=B
                    )[:, m0:m0 + 4, b:b + 1, ci * 128:(ci + 1) * 128]
                    nc.vector.tensor_tensor(
                        dst, src, rbv, op=MULT,
                    )
                else:
                    dst = stg[:].rearrange(
                        "p (h q) -> p h q", h=4)[:, :, ci * 128:(ci + 1) * 128]
                    nc.vector.tensor_tensor(
                        dst, src, rbv, op=MULT,
                    )
            if kb:
                nc.sync.dma_start(
                    yt[64:128].rearrange(
                        "p (m b q) -> p m b q", m=8, b=B
                    )[:, m0:m0 + 4, b:b + 1, :],
                    stg[:].rearrange("p (h q) -> p h q", h=4),
                )

        def emit_O(b, mq2, nk, copy_eng="act"):
            po = psP.tile([128, 1024], F32, tag="s", name=f"po{b}{mq2}{nk}")
            for m in range(8):
                nc.tensor.matmul(
                    po[:, 0:512],
                    yt[:, m * (B * QB) + b * QB + mq2 * 128:
                       m * (B * QB) + b * QB + (mq2 + 1) * 128],
                    wo[m][:, nk * 512:(nk + 1) * 512],
                    start=(m == 0), stop=(m == 7),
                )
            ost = misc.tile([128, 512], F32, tag="ost", name=f"o{b}{mq2}{nk}")
            if copy_eng == "act":
                nc.scalar.copy(ost[:], po[:, 0:512])
            else:
                nc.vector.tensor_copy(ost[:], po[:, 0:512])
            nc.sync.dma_start(
                out_d[b, mq2 * 128:(mq2 + 1) * 128, nk * 512:(nk + 1) * 512],
                ost[:],
            )

        # ------------------------------------------------ main schedule
        emit_transposes(0)
        emit_K(0)
        emit_V(0)
        emit_Q(0)

        # Software-pipelined attention: per step emit norm(i-2), S(i),
        # PV(i-1) so the PE never waits on the exp->mask chain.  kb=64
        # groups first so the per-batch tail writes yt directly (no staging
        # DMA on the critical tail).
        GORDER = [1, 3, 0, 2]
        iters = [(g, 0) for g in GORDER] + [(g, 1) for g in GORDER]
        fillers = {0: [lambda: emit_transposes(1)],
                   1: [lambda: emit_K(1)],
                   2: [lambda: emit_V(1, (0, 1), sink=False)],
                   3: [lambda: emit_Q(1)],
                   4: [lambda: emit_V(1, (2, 3), sink=True)],
                   5: [lambda: emit_O(0, 0, 0)],
                   6: [lambda: emit_O(0, 0, 1), lambda: emit_O(0, 1, 0)],
                   7: [lambda: emit_O(0, 1, 1)]}
        for i, (g, b) in enumerate(iters):
            if i >= 2:
                emit_attn_norm(*iters[i - 2])
            emit_attn_S(g, b)
            if i >= 1:
                emit_attn_PV(*iters[i - 1])
            for f in fillers.get(i, []):
                f()
        emit_attn_norm(*iters[6])
        emit_attn_PV(*iters[7])
        emit_attn_norm(*iters[7])

        for oi, (mq2, nk) in enumerate([(0, 0), (0, 1), (1, 0), (1, 1)]):
            emit_O(1, mq2, nk, copy_eng="act" if oi % 2 == 0 else "dve")

    nc.compile()
    return nc


# ================================================================ host side
def host_prep(X, Wq, Wk, Wv, Wo):
    """Returns in_maps (list of per-core dicts of numpy arrays)."""
    import ml_dtypes
    bf16 = np.dtype(ml_dtypes.bfloat16)

    X = np.asarray(X, dtype=np.float32)
    Wq = np.asarray(Wq, dtype=np.float32)
    Wk = np.asarray(Wk, dtype=np.float32)
    Wv = np.asarray(Wv, dtype=np.float32)
    Wo = np.asarray(Wo, dtype=np.float32)

    flat_perm = np.concatenate(
        [np.arange(h * HD, (h + 1) * HD) for h in HEAD_ORDER]
    )
    wq_p = (np.ascontiguousarray(Wq[:, flat_perm])
            * np.float32(1.0 / np.sqrt(HD))).astype(bf16)
    wo_p = np.ascontiguousarray(Wo[flat_perm, :]).astype(bf16)
    wk_c = Wk.astype(bf16)
    wv_c = Wv.astype(bf16)

    tt = np.arange(T)
    i = tt[:, None]
    j = tt[None, :]
    m_full = ((j <= i) & ((j < SINK) | (j >= np.maximum(i - WIN + 1, 0)))
              ).astype(np.float32)

    X16 = X.astype(bf16)
    xs = np.ascontiguousarray(X16[:, 0:SINK, :])
    idt = np.eye(128, dtype=np.float32)
    onbm = np.ones((65, 64), dtype=np.float32)

    in_maps = []
    for c in range(NCORES):
        qs = c * QB
        ks = qs - QB

        xw = np.zeros((B, KW, D), dtype=bf16)
        lo = max(ks, 0)
        xw[:, lo - ks:, :] = X16[:, lo:ks + KW, :]

        def wblk(tki, ch):
            jg = ks + tki * 128 + np.arange(128)
            ig = qs + ch * 128 + np.arange(128)
            msk = np.zeros((128, 128), dtype=np.float32)
            vr = jg >= 0
            if vr.any():
                msk[vr, :] = m_full[np.ix_(ig, jg[vr])].T
            return np.tile(msk, (1, 4))

        mpv = np.stack([wblk(0, 0), wblk(1, 1), wblk(2, 0), wblk(3, 1)])

        mts = np.zeros((SINK, QB), dtype=np.float32)
        for jj in range(SINK):
            for ch in range(2):
                cov_lo = ks + ch * 128
                cov_hi = cov_lo + 384
                if cov_lo <= jj < cov_hi:
                    continue  # delivered by window blocks
                mm_ = m_full[qs + ch * 128:qs + (ch + 1) * 128, jj]
                mts[jj, ch * 128:(ch + 1) * 128] = mm_
        msnk = np.concatenate(
            [np.tile(mts[:, 0:128], (1, 4)), np.tile(mts[:, 128:256], (1, 4))],
            axis=1,
        )

        onew = np.zeros((4, 128, NKV), dtype=np.float32)
        for tki in range(4):
            real = (ks + tki * 128 + np.arange(128)) >= 0
            onew[tki, real, :] = 1.0

        in_maps.append({
            "Xw": xw,
            "Xs": xs,
            "IDT": idt.astype(bf16),
            "Wq": wq_p,
            "Wk": wk_c,
            "Wv": wv_c,
            "Wo": wo_p,
            "MP": mpv.astype(bf16),
            "MSNK": msnk.astype(bf16),
            "ONEW": onew.astype(bf16),
            "ONB": onbm,
        })
    return in_maps


_NC_CACHE = {}


def get_nc():
    if "nc" not in _NC_CACHE:
        _NC_CACHE["nc"] = build_nc()
    return _NC_CACHE["nc"]


def kernel(X, Wq, Wk, Wv, Wo):
    in_maps = host_prep(X, Wq, Wk, Wv, Wo)
    nc = get_nc()
    res = run_bass_kernel_spmd(nc, in_maps, list(range(NCORES)))
    out = np.empty((B, T, D), dtype=np.float32)
    for c in range(NCORES):
        out[:, c * QB:(c + 1) * QB, :] = res.results[c]["out"]
    return out
